# revision 62
# baseline (speedup 1.0000x reference)
"""GQA attention layer (B=2, S=2048, H=2048, 16 q heads / 4 kv heads, RoPE, causal
mask) on 8 TRN2 NeuronCores.

Sharding: core c = (b, g) with b = c // 4 (batch), g = c % 4 (kv-head group).
Each core computes q-heads 4g..4g+3 and kv-head g for batch b:
  - QKV projections from x^T (bf16 matmuls, f32 PSUM accumulate)
  - RoPE on q/k (f32, host-precomputed cos/sin tables); v transposed via PE
  - MM1 computes TRANSPOSED scores S^T[k, q] (lhsT = k^T tile, rhs = q^T), so
    exp on ScalarE writes P^T directly -- no DMA transposes anywhere (Tile
    hard-serializes DMA transposes against collectives, which would freeze
    the pipeline during every AllGather)
  - row-sums of P: groups of 4 k-tiles combined on VectorE, then a
    ones-vector matmul; 1/sum applied to ctx^T via GpSimd partition
    broadcast + VectorE multiply
  - MM2: ctx^T[d, q] = sum_kt v[kt]^T-block @ P^T[kt]
  - AllGather ctx^T across the 4 cores of the batch (groups [[0..3],[4..7]])
  - o-proj: each core computes its 512-column block of the output for all S.
Host reassembles [B, S, H] from the 8 [S, 512] column blocks.

The additive attention mask is handled generally: each 512q x 128k score tile
is classified at build time (from the actual mask input) as fully-masked
(matmuls skipped), zero (no mask op), or mixed (P^T is multiplied post-exp
by a per-core transposed exp(mask) tile). For the causal mask this skips the upper
triangle (~half the attention FLOPs) and needs only 4 distinct patterns.

Scheduling: the attention chain (MM1 -> exp -> sums -> MM2) is latency- and
ScalarE-bound; independent matmul work (QKV projection of later chunks,
o-proj of earlier chunks) is fed into the emission stream at single-matmul
granularity between the dependent attention matmuls, keeping TensorE dense.
AllGather-gated DMA (gathered-ctx loads, output writes) lives on the GpSimd
SWDGE queue so it cannot head-of-line-block compute-critical DMA queues.
"""
import math
import os
import sys

for _p in ("/opt/trn_rl_repo",):
    if _p not in sys.path and os.path.isdir(_p):
        sys.path.insert(0, _p)

import ml_dtypes
import numpy as np

from concourse import bacc, mybir, tile
from concourse.bass_utils import run_bass_kernel_spmd

BF16 = mybir.dt.bfloat16
F32 = mybir.dt.float32
EXP = mybir.ActivationFunctionType.Exp

B, S, H = 2, 2048, 2048
NH, NKV, HD = 16, 4, 128
GQ = NH // NKV            # q heads per core (4)
DL = GQ * HD              # local q width (512)
ET = H // 128             # e-tiles (16)
NSC = S // 512            # 512-wide s/q chunks (4)
NKT = S // 128            # 128-row k tiles (16)
ROPE_THETA = 10000.0
INV_SQRT_HD = 1.0 / math.sqrt(HD)
SQRT_HD = math.sqrt(HD)

SKIP, FREE = 0, 1         # vis codes; >=2 means pattern index (code - 2)

_cache = {}


def _classify_mask(mask):
    """mask: [B, 1, S, S] f32 additive. Tiles are [512 q x 128 k] per
    (chunk c, k-tile kt). Returns (vis, pats): vis[c][kt] in
    {SKIP, FREE, idx+2}; pats[b] is [n_pat, 128, 512] f32 — TRANSPOSED
    [k, q] patterns, clipped and pre-multiplied by sqrt(HD)."""
    m = np.asarray(mask, dtype=np.float32).reshape(B, S, S)
    vis = [[FREE] * NKT for _ in range(NSC)]
    pat_ids = {}
    pats = [[] for _ in range(B)]
    for c in range(NSC):
        for kt in range(NKT):
            blk = m[:, c * 512:(c + 1) * 512, kt * 128:(kt + 1) * 128]
            if np.all(blk <= -1e8):
                vis[c][kt] = SKIP
            elif np.all(blk == 0.0):
                vis[c][kt] = FREE
            else:
                # multiplicative post-exp pattern (transposed [k, q]):
                # exp(s + m) == exp(s) * exp(m)
                factor = np.exp(np.maximum(blk, -90.0)).transpose(0, 2, 1)
                factor = np.ascontiguousarray(factor)
                key = factor.tobytes()
                if key not in pat_ids:
                    pat_ids[key] = len(pats[0])
                    for b in range(B):
                        pats[b].append(factor[b])
                vis[c][kt] = pat_ids[key] + 2
    n_pat = len(pats[0])
    if n_pat == 0:
        pats_np = [np.zeros((1, 128, 512), np.float32) for _ in range(B)]
        n_pat = 1
    else:
        pats_np = [np.stack(p) for p in pats]
    return vis, pats_np, n_pat


class Feeder:
    """FIFO pull of single-matmul filler steps from generators."""

    def __init__(self):
        self.gens = []

    def add(self, gen):
        self.gens.append(gen)

    def pull(self, n):
        while n > 0 and self.gens:
            try:
                next(self.gens[0])
                n -= 1
            except StopIteration:
                self.gens.pop(0)

    def drain(self):
        while self.gens:
            self.pull(1 << 30)


def _build(vis, n_pat):
    nc = bacc.Bacc(None, target_bir_lowering=False, num_devices=8)

    xT_d = nc.dram_tensor("xT", [H, S], BF16, kind="ExternalInput")
    wqT_d = nc.dram_tensor("wqT", [H, DL], BF16, kind="ExternalInput")
    wkT_d = nc.dram_tensor("wkT", [H, HD], BF16, kind="ExternalInput")
    wvT_d = nc.dram_tensor("wvT", [H, HD], BF16, kind="ExternalInput")
    woT_d = nc.dram_tensor("woT", [H, DL], BF16, kind="ExternalInput")
    cosT_d = nc.dram_tensor("cosT", [HD, S], F32, kind="ExternalInput")
    sinT_d = nc.dram_tensor("sinT", [HD, S], F32, kind="ExternalInput")
    pats_d = nc.dram_tensor("pats", [n_pat, 128, 512], BF16, kind="ExternalInput")
    ident_d = nc.dram_tensor("ident", [128, 128], BF16, kind="ExternalInput")
    out_d = nc.dram_tensor("out", [S, DL], F32, kind="ExternalOutput")

    vis_kts = [[kt for kt in range(NKT) if vis[c][kt] != SKIP] for c in range(NSC)]

    with tile.TileContext(nc) as tc:
        with (
            tc.tile_pool(name="wp", bufs=1) as wp,
            tc.tile_pool(name="xp", bufs=2) as xp,
            tc.tile_pool(name="qk", bufs=1) as qk,
            tc.tile_pool(name="rp", bufs=2) as rp,
            tc.tile_pool(name="ptp", bufs=2) as ptp,
            tc.tile_pool(name="cp", bufs=2) as cp,
            tc.tile_pool(name="fp", bufs=1) as fp,
            tc.tile_pool(name="op", bufs=2) as op,
            tc.tile_pool(name="sp", bufs=4) as sp,
            tc.tile_pool(name="ps", bufs=1, space="PSUM") as psp,
            tc.tile_pool(name="dr", bufs=2, space="DRAM") as dr,
        ):
            # ---- weights / tables for phase 1 ----
            wq_sb = [wp.tile([128, DL], BF16, name=f"wq{e}") for e in range(ET)]
            wk_sb = [wp.tile([128, HD], BF16, name=f"wk{e}") for e in range(ET)]
            wv_sb = [wp.tile([128, HD], BF16, name=f"wv{e}") for e in range(ET)]
            cos_sb = wp.tile([HD, S], F32, name="cos_sb")
            sin_sb = wp.tile([HD, S], F32, name="sin_sb")
            nc.scalar.dma_start(cos_sb[:, 0:512], cosT_d[:, 0:512])
            nc.scalar.dma_start(sin_sb[:, 0:512], sinT_d[:, 0:512])
            xts0 = []
            for e in range(ET):
                r = slice(e * 128, (e + 1) * 128)
                nc.sync.dma_start(wq_sb[e][:], wqT_d[r, :])
                t = xp.tile([128, 512], BF16, name=f"xts{e}")
                (nc.sync if e % 2 == 0 else nc.scalar).dma_start(
                    t[:], xT_d[r, 0:512])
                xts0.append(t)
                nc.scalar.dma_start(wk_sb[e][:], wkT_d[r, :])
                nc.scalar.dma_start(wv_sb[e][:], wvT_d[r, :])
            ident_sb = wp.tile([128, 128], BF16, name="ident_sb")
            nc.scalar.dma_start(ident_sb[:], ident_d[:])
            ones_sb = wp.tile([128, 1], BF16, name="ones_sb")
            nc.vector.memset(ones_sb[:], 1.0)

            # ---- persistent activations ----
            qT = [qk.tile([HD, S], BF16, name=f"qT{h}") for h in range(GQ)]
            kT = qk.tile([HD, S], BF16, name="kT")
            v_all = qk.tile([128, NKT, HD], BF16, name="v_all")
            wo_sb = [wp.tile([128, DL], BF16, name=f"wo{e}") for e in range(ET)]
            pat_sb = [wp.tile([128, 512], BF16, name=f"pat{i}")
                      for i in range(n_pat)]

            def rope(ps, out_slice, sc):
                cs = slice(sc * 512, (sc + 1) * 512)
                t1 = rp.tile([128, 512], F32, name="rope_t1")
                nc.vector.tensor_mul(t1[0:64, :], ps[64:128, :], sin_sb[0:64, cs])
                nc.vector.tensor_mul(t1[64:128, :], ps[0:64, :], sin_sb[64:128, cs])
                t2 = rp.tile([128, 512], F32, name="rope_t2")
                nc.vector.tensor_mul(t2[:], ps[:], cos_sb[:, cs])
                nc.vector.tensor_add(out_slice, t2[:], t1[:])

            def preload_x(sc):
                cs = slice(sc * 512, (sc + 1) * 512)
                xts = []
                for e in range(ET):
                    t = xp.tile([128, 512], BF16, name=f"xts{e}")
                    nc.sync.dma_start(t[:], xT_d[e * 128:(e + 1) * 128, cs])
                    xts.append(t)
                return xts

            def qkv_gen(sc, xts=None):
                """Yields once per matmul; 6 groups (4 q heads, k, v)."""
                cs = slice(sc * 512, (sc + 1) * 512)
                if xts is None:
                    xts = preload_x(sc)

                def fin_q(h):
                    return lambda ps: rope(ps, qT[h][:, cs], sc)

                def fin_k(ps):
                    rope(ps, kT[:, cs], sc)

                def fin_v(ps):
                    vt = rp.tile([128, 512], BF16, name="vt_tmp")
                    nc.vector.tensor_copy(vt[:], ps[:])
                    for i in range(4):
                        vtp = psp.tile([128, 128], BF16, name="vtp_ps",
                                       tag="sum_ps", bufs=1)
                        nc.tensor.transpose(vtp[:], vt[:, i * 128:(i + 1) * 128],
                                            ident_sb[:])
                        nc.vector.tensor_copy(v_all[:, sc * 4 + i, :], vtp[:])

                def blk_q(h):
                    return (lambda e, h=h: wq_sb[e][:, h * 128:(h + 1) * 128],
                            fin_q(h))

                if sc == 0:
                    blocks = [blk_q(0), (lambda e: wk_sb[e][:], fin_k),
                              blk_q(1), (lambda e: wv_sb[e][:], fin_v),
                              blk_q(2), blk_q(3)]
                else:
                    blocks = [blk_q(0), blk_q(1),
                              (lambda e: wk_sb[e][:], fin_k),
                              (lambda e: wv_sb[e][:], fin_v),
                              blk_q(2), blk_q(3)]
                for lhs_fn, fin in blocks:
                    ps = psp.tile([128, 512], F32, name="fill_ps", bufs=2)
                    for e in range(ET):
                        nc.tensor.matmul(ps[:], lhs_fn(e), xts[e][:],
                                         start=(e == 0), stop=(e == ET - 1))
                        yield
                    fin(ps)

            def oproj_gen(c, ag_outs):
                """Yields once per matmul; 4 groups of 16 (o-proj chunk c).
                Gathered-ctx loads are emitted lazily at the first pull —
                by then (two chunks after the AllGather was issued) the
                collective is long done, so the loads cannot head-of-line
                block the sync DMA queue."""
                tiles = []
                order = []
                g = 0
                for ag_out, itiles in ag_outs:
                    for r0 in range(0, len(itiles), 4):
                        t = fp.tile([128, 4, 512], BF16, name=f"ctxF{g}")
                        nc.sync.dma_start(
                            t[:], ag_out[r0 * 128:(r0 + 4) * 128, :].rearrange(
                                "(a p) f -> p a f", p=128))
                        tiles.append(t)
                        for j in range(4):
                            order.append((g, j, itiles[r0 + j]))
                        g += 1
                for qs in range(4):
                    ops = psp.tile([128, 512], F32, name="fill_ps", bufs=2)
                    for j, (g, jj, i) in enumerate(order):
                        nc.tensor.matmul(
                            ops[:],
                            tiles[g][:, jj, qs * 128:(qs + 1) * 128],
                            wo_sb[i][:],
                            start=(j == 0), stop=(j == ET - 1))
                        yield
                    osb = op.tile([128, 512], F32, name="osb")
                    nc.vector.tensor_copy(osb[:], ops[:])
                    r0 = c * 512 + qs * 128
                    nc.sync.dma_start(out_d[r0:r0 + 128, :], osb[:])

            carry = []

            def emit_attn(c, feeder, n_fill, split_ag=False, defer_tail=False):
                """Attention for q chunk c in transposed-score form. With
                defer_tail, the last head's sums/MM2 and the AllGather issue
                are executed early in the NEXT chunk's emission so the chunk
                boundary never stalls TensorE."""
                kts = vis_kts[c]
                cs = slice(c * 512, (c + 1) * 512)
                n_attn_mm = len(kts) * GQ * 3  # MM1T + ones + MM2 per (h, kt)
                ratio = n_fill / max(1, n_attn_mm)
                acc = 0.0

                def tick(k=1):
                    nonlocal acc
                    acc += ratio * k
                    n = int(acc)
                    if n:
                        feeder.pull(n)
                        acc -= n

                if split_ag:
                    ag_ins = [dr.tile([2 * HD, 512], BF16, name="ag_in_a"),
                              dr.tile([2 * HD, 512], BF16, name="ag_in_b")]
                else:
                    ag_ins = [dr.tile([DL, 512], BF16, name="ag_in")]
                ags = []

                def issue_ag(part):
                    ag_in = ag_ins[part]
                    nr = ag_in.shape[0]
                    ag_out = dr.tile([4 * nr, 512], BF16,
                                     name=f"ag_out{'ab'[part] if split_ag else ''}")
                    nc.gpsimd.collective_compute(
                        "AllGather", mybir.AluOpType.bypass,
                        replica_groups=[[0, 1, 2, 3], [4, 5, 6, 7]],
                        ins=[ag_in[:].opt()], outs=[ag_out[:].opt()])
                    nh_part = nr // HD
                    itiles = [4 * r + part * nh_part + h
                              for r in range(4) for h in range(nh_part)]
                    ags.append((ag_out, itiles))

                done_mm2 = [0]
                pending = []
                for h in range(GQ):
                    PT = ptp.tile([128, NKT, 512], BF16, name="PT")
                    ksums = []
                    flush_carry = carry and h == 0
                    # MM1T per visible k-tile; exp over adjacent-kt pairs
                    for i0 in range(0, len(kts), 2):
                        pair = kts[i0:i0 + 2]
                        ps = psp.tile([128, 1024], F32, name="mm1_ps", bufs=2)
                        for j, kt in enumerate(pair):
                            nc.tensor.matmul(ps[:, j * 512:(j + 1) * 512],
                                             kT[:, kt * 128:(kt + 1) * 128],
                                             qT[h][:, cs])
                            tick()
                        if len(pair) == 2 and pair[1] == pair[0] + 1:
                            nc.scalar.activation(
                                PT[:, pair[0]:pair[0] + 2, :],
                                ps[:, 0:1024], EXP, scale=INV_SQRT_HD)
                        else:
                            for j, kt in enumerate(pair):
                                nc.scalar.activation(
                                    PT[:, kt, :], ps[:, j * 512:(j + 1) * 512],
                                    EXP, scale=INV_SQRT_HD)
                        for kt in pair:
                            code = vis[c][kt]
                            if code >= 2:
                                nc.vector.tensor_mul(PT[:, kt, :], PT[:, kt, :],
                                                     pat_sb[code - 2][:])
                        # pre-combine groups of 4 k-tiles for the row sums on
                        # VectorE; overlaps the later MM1T pairs on TensorE
                        if i0 % 8 == 0:
                            ksums.append([None, kts[i0:i0 + 8]])
                            if len(pair) == 2:
                                ksum = sp.tile([128, 512], BF16, name="ktsum",
                                               bufs=6)
                                nc.vector.tensor_add(ksum[:], PT[:, pair[0], :],
                                                     PT[:, pair[1], :])
                                ksums[-1][0] = ksum
                        else:
                            ksum = ksums[-1][0]
                            for kt in pair:
                                nc.vector.tensor_add(ksum[:], ksum[:],
                                                     PT[:, kt, :])

                    def finish(h=h, PT=PT, hks=list(ksums)):
                        # row sums via ones-matmul per combined group, then
                        # 1/sum broadcast; MM2; normalize; AllGather input
                        sum_ps = psp.tile([1, 512], F32, name="sum_ps",
                                          tag="sum_ps", bufs=1)
                        for gi, (ksum, grp) in enumerate(hks):
                            rhs = ksum[:] if ksum is not None else PT[:, grp[0], :]
                            nc.tensor.matmul(sum_ps[:], ones_sb[:], rhs,
                                             start=(gi == 0),
                                             stop=(gi == len(hks) - 1))
                            tick()
                        rrow = sp.tile([1, 512], F32, name="rrow", bufs=2)
                        nc.vector.reciprocal(rrow[:], sum_ps[:])
                        rbc = sp.tile([128, 512], F32, name="rbc", bufs=2)
                        nc.gpsimd.partition_broadcast(rbc[:], rrow[0:1, :])
                        ctx_ps = psp.tile([128, 512], F32, name="ctx_ps",
                                          tag="ctx_ps", bufs=1)
                        for j, kt in enumerate(kts):
                            nc.tensor.matmul(ctx_ps[:], v_all[:, kt, :],
                                             PT[:, kt, :], start=(j == 0),
                                             stop=(j == len(kts) - 1))
                            tick()
                        ctx_sb = cp.tile([128, 512], BF16, name=f"ctx{h}")
                        nc.vector.tensor_mul(ctx_sb[:], ctx_ps[:], rbc[:])
                        if split_ag:
                            ag_in, row = ag_ins[h // 2], (h % 2) * 128
                        else:
                            ag_in, row = ag_ins[0], h * 128
                        nc.sync.dma_start(ag_in[row:row + 128, :], ctx_sb[:])
                        done_mm2[0] += 1
                        if split_ag and done_mm2[0] == 2:
                            issue_ag(0)

                    if flush_carry:
                        carry.pop(0)()
                    pending.append(finish)
                    if len(pending) > 1:
                        pending.pop(0)()
                feeder.pull(8)
                if defer_tail and not split_ag:
                    last = pending.pop()

                    def tail(last=last):
                        last()
                        issue_ag(0)
                    carry.append(tail)
                else:
                    while pending:
                        pending.pop(0)()
                    if split_ag:
                        issue_ag(1)
                    else:
                        issue_ag(0)
                return ags

            # ---- pipelined emission ----
            f0 = Feeder()
            f0.add(qkv_gen(0, xts0))
            f0.drain()
            for sc in range(1, NSC):
                cs = slice(sc * 512, (sc + 1) * 512)
                nc.scalar.dma_start(cos_sb[:, cs], cosT_d[:, cs])
                nc.scalar.dma_start(sin_sb[:, cs], sinT_d[:, cs])
            for i in range(n_pat):
                nc.scalar.dma_start(pat_sb[i][:], pats_d[i])
            for e in range(ET):
                nc.scalar.dma_start(wo_sb[e][:], woT_d[e * 128:(e + 1) * 128, :])

            fd = Feeder()
            fd.add(qkv_gen(1))
            fd.add(qkv_gen(2))
            ags0 = emit_attn(0, fd, 192, defer_tail=True)
            fd.add(qkv_gen(3))
            ags1 = emit_attn(1, fd, 96)
            fd.add(oproj_gen(0, ags0))
            ags2 = emit_attn(2, fd, 64)
            fd.add(oproj_gen(1, ags1))
            fd.add(oproj_gen(2, ags2))
            ags3 = emit_attn(3, fd, 128, split_ag=True)
            fd.add(oproj_gen(3, ags3))
            fd.drain()

    nc.compile()
    return nc


def _prepare(hidden_states, wq, wk, wv, wo, attention_mask, position_ids):
    hidden_states = np.asarray(hidden_states, dtype=np.float32)
    wq = np.asarray(wq, dtype=np.float32)
    wk = np.asarray(wk, dtype=np.float32)
    wv = np.asarray(wv, dtype=np.float32)
    wo = np.asarray(wo, dtype=np.float32)
    pos = np.asarray(position_ids)

    vis, pats, n_pat = _classify_mask(attention_mask)
    key = (tuple(tuple(r) for r in vis), n_pat)
    if key not in _cache:
        _cache[key] = _build(vis, n_pat)
    nc = _cache[key]

    # RoPE tables per batch: cosT/sinT [HD, S]; sinT sign-folded (-sin for d<64)
    inv_freq = 1.0 / (ROPE_THETA ** (np.arange(0, HD, 2, dtype=np.float32) / HD))
    cosT = np.empty((B, HD, S), np.float32)
    sinT = np.empty((B, HD, S), np.float32)
    for b in range(B):
        freqs = pos[b].astype(np.float32)[None, :] * inv_freq[:, None]  # [64, S]
        cosT[b] = np.concatenate([np.cos(freqs)] * 2, axis=0)
        sn = np.sin(freqs)
        sinT[b] = np.concatenate([-sn, sn], axis=0)

    bf = ml_dtypes.bfloat16
    xT = [np.ascontiguousarray(hidden_states[b].T).astype(bf) for b in range(B)]
    ident = np.eye(128, dtype=np.float32).astype(bf)
    in_maps = []
    for c in range(8):
        b, g = c // 4, c % 4
        in_maps.append({
            "xT": xT[b],
            "wqT": np.ascontiguousarray(wq[g * DL:(g + 1) * DL, :].T).astype(bf),
            "wkT": np.ascontiguousarray(wk[g * HD:(g + 1) * HD, :].T).astype(bf),
            "wvT": np.ascontiguousarray(wv[g * HD:(g + 1) * HD, :].T).astype(bf),
            "woT": np.ascontiguousarray(wo[g * DL:(g + 1) * DL, :].T).astype(bf),
            "cosT": cosT[b],
            "sinT": sinT[b],
            "pats": pats[b].astype(bf),
            "ident": ident,
        })
    return nc, in_maps


def kernel(hidden_states, wq, wk, wv, wo, attention_mask, position_ids):
    nc, in_maps = _prepare(hidden_states, wq, wk, wv, wo, attention_mask,
                           position_ids)
    res = run_bass_kernel_spmd(nc, in_maps, core_ids=list(range(8))).results
    out = np.empty((B, S, H), np.float32)
    for c in range(8):
        b, g = c // 4, c % 4
        out[b, :, g * DL:(g + 1) * DL] = res[c]["out"]
    return out


# revision 63
# speedup vs baseline: 1.0196x; 1.0196x over previous
"""GQA attention layer (B=2, S=2048, H=2048, 16 q heads / 4 kv heads, RoPE, causal
mask) on 8 TRN2 NeuronCores.

Sharding: core c = (b, g) with b = c // 4 (batch), g = c % 4 (kv-head group).
Each core computes q-heads 4g..4g+3 and kv-head g for batch b:
  - QKV projections from x^T (bf16 matmuls, f32 PSUM accumulate)
  - RoPE on q/k (f32, host-precomputed cos/sin tables); v transposed via PE
  - MM1 computes TRANSPOSED scores S^T[k, q] (lhsT = k^T tile, rhs = q^T), so
    exp on ScalarE writes P^T directly -- no DMA transposes anywhere (Tile
    hard-serializes DMA transposes against collectives, which would freeze
    the pipeline during every AllGather)
  - row-sums of P: groups of 4 k-tiles combined on VectorE, then a
    ones-vector matmul; 1/sum applied to ctx^T via GpSimd partition
    broadcast + VectorE multiply
  - MM2: ctx^T[d, q] = sum_kt v[kt]^T-block @ P^T[kt]
  - AllGather ctx^T across the 4 cores of the batch (groups [[0..3],[4..7]])
  - o-proj: each core computes its 512-column block of the output for all S.
Host reassembles [B, S, H] from the 8 [S, 512] column blocks.

The additive attention mask is handled generally: each 512q x 128k score tile
is classified at build time (from the actual mask input) as fully-masked
(matmuls skipped), zero (no mask op), or mixed (P^T is multiplied post-exp
by a per-core transposed exp(mask) tile). For the causal mask this skips the upper
triangle (~half the attention FLOPs) and needs only 4 distinct patterns.

Scheduling: the attention chain (MM1 -> exp -> sums -> MM2) is latency- and
ScalarE-bound; independent matmul work (QKV projection of later chunks,
o-proj of earlier chunks) is fed into the emission stream at single-matmul
granularity between the dependent attention matmuls, keeping TensorE dense.
AllGather-gated DMA (gathered-ctx loads, output writes) lives on the GpSimd
SWDGE queue so it cannot head-of-line-block compute-critical DMA queues.
"""
import math
import os
import sys

for _p in ("/opt/trn_rl_repo",):
    if _p not in sys.path and os.path.isdir(_p):
        sys.path.insert(0, _p)

import ml_dtypes
import numpy as np

from concourse import bacc, mybir, tile
from concourse.bass_utils import run_bass_kernel_spmd

BF16 = mybir.dt.bfloat16
F32 = mybir.dt.float32
EXP = mybir.ActivationFunctionType.Exp

B, S, H = 2, 2048, 2048
NH, NKV, HD = 16, 4, 128
GQ = NH // NKV            # q heads per core (4)
DL = GQ * HD              # local q width (512)
ET = H // 128             # e-tiles (16)
NSC = S // 512            # 512-wide s/q chunks (4)
NKT = S // 128            # 128-row k tiles (16)
ROPE_THETA = 10000.0
INV_SQRT_HD = 1.0 / math.sqrt(HD)
SQRT_HD = math.sqrt(HD)

SKIP, FREE = 0, 1         # vis codes; >=2 means pattern index (code - 2)

_cache = {}


def _classify_mask(mask):
    """mask: [B, 1, S, S] f32 additive. Tiles are [512 q x 128 k] per
    (chunk c, k-tile kt). Returns (vis, pats): vis[c][kt] in
    {SKIP, FREE, idx+2}; pats[b] is [n_pat, 128, 512] f32 — TRANSPOSED
    [k, q] patterns, clipped and pre-multiplied by sqrt(HD)."""
    m = np.asarray(mask, dtype=np.float32).reshape(B, S, S)
    vis = [[FREE] * NKT for _ in range(NSC)]
    pat_ids = {}
    pats = [[] for _ in range(B)]
    for c in range(NSC):
        for kt in range(NKT):
            blk = m[:, c * 512:(c + 1) * 512, kt * 128:(kt + 1) * 128]
            if np.all(blk <= -1e8):
                vis[c][kt] = SKIP
            elif np.all(blk == 0.0):
                vis[c][kt] = FREE
            else:
                # multiplicative post-exp pattern (transposed [k, q]):
                # exp(s + m) == exp(s) * exp(m)
                factor = np.exp(np.maximum(blk, -90.0)).transpose(0, 2, 1)
                factor = np.ascontiguousarray(factor)
                key = factor.tobytes()
                if key not in pat_ids:
                    pat_ids[key] = len(pats[0])
                    for b in range(B):
                        pats[b].append(factor[b])
                vis[c][kt] = pat_ids[key] + 2
    n_pat = len(pats[0])
    if n_pat == 0:
        pats_np = [np.zeros((1, 128, 512), np.float32) for _ in range(B)]
        n_pat = 1
    else:
        pats_np = [np.stack(p) for p in pats]
    return vis, pats_np, n_pat


class Feeder:
    """FIFO pull of single-matmul filler steps from generators."""

    def __init__(self):
        self.gens = []

    def add(self, gen):
        self.gens.append(gen)

    def pull(self, n):
        while n > 0 and self.gens:
            try:
                next(self.gens[0])
                n -= 1
            except StopIteration:
                self.gens.pop(0)

    def drain(self):
        while self.gens:
            self.pull(1 << 30)


def _build(vis, n_pat):
    nc = bacc.Bacc(None, target_bir_lowering=False, num_devices=8)

    xT_d = nc.dram_tensor("xT", [H, S], BF16, kind="ExternalInput")
    wqT_d = nc.dram_tensor("wqT", [H, DL], BF16, kind="ExternalInput")
    wkT_d = nc.dram_tensor("wkT", [H, HD], BF16, kind="ExternalInput")
    wvT_d = nc.dram_tensor("wvT", [H, HD], BF16, kind="ExternalInput")
    woT_d = nc.dram_tensor("woT", [H, DL], BF16, kind="ExternalInput")
    cosT_d = nc.dram_tensor("cosT", [HD, S], F32, kind="ExternalInput")
    sinT_d = nc.dram_tensor("sinT", [HD, S], F32, kind="ExternalInput")
    pats_d = nc.dram_tensor("pats", [n_pat, 128, 512], BF16, kind="ExternalInput")
    ident_d = nc.dram_tensor("ident", [128, 128], BF16, kind="ExternalInput")
    out_d = nc.dram_tensor("out", [S, DL], F32, kind="ExternalOutput")

    vis_kts = [[kt for kt in range(NKT) if vis[c][kt] != SKIP] for c in range(NSC)]

    with tile.TileContext(nc) as tc:
        with (
            tc.tile_pool(name="wp", bufs=1) as wp,
            tc.tile_pool(name="xp", bufs=2) as xp,
            tc.tile_pool(name="qk", bufs=1) as qk,
            tc.tile_pool(name="rp", bufs=2) as rp,
            tc.tile_pool(name="ptp", bufs=2) as ptp,
            tc.tile_pool(name="cp", bufs=2) as cp,
            tc.tile_pool(name="fp", bufs=1) as fp,
            tc.tile_pool(name="op", bufs=2) as op,
            tc.tile_pool(name="sp", bufs=4) as sp,
            tc.tile_pool(name="ps", bufs=1, space="PSUM") as psp,
            tc.tile_pool(name="dr", bufs=2, space="DRAM") as dr,
        ):
            # ---- weights / tables for phase 1 ----
            wq_sb = [wp.tile([128, DL], BF16, name=f"wq{e}") for e in range(ET)]
            wk_sb = [wp.tile([128, HD], BF16, name=f"wk{e}") for e in range(ET)]
            wv_sb = [wp.tile([128, HD], BF16, name=f"wv{e}") for e in range(ET)]
            cos_sb = wp.tile([HD, S], F32, name="cos_sb")
            sin_sb = wp.tile([HD, S], F32, name="sin_sb")
            nc.scalar.dma_start(cos_sb[:, 0:512], cosT_d[:, 0:512])
            nc.scalar.dma_start(sin_sb[:, 0:512], sinT_d[:, 0:512])
            xts0 = []
            for e in range(ET):
                r = slice(e * 128, (e + 1) * 128)
                nc.sync.dma_start(wq_sb[e][:], wqT_d[r, :])
                t = xp.tile([128, 512], BF16, name=f"xts{e}")
                nc.sync.dma_start(t[:], xT_d[r, 0:512])
                xts0.append(t)
            for e in range(ET):
                r = slice(e * 128, (e + 1) * 128)
                nc.scalar.dma_start(wk_sb[e][:], wkT_d[r, :])
                nc.scalar.dma_start(wv_sb[e][:], wvT_d[r, :])
            ident_sb = wp.tile([128, 128], BF16, name="ident_sb")
            nc.scalar.dma_start(ident_sb[:], ident_d[:])
            ones_sb = wp.tile([128, 1], BF16, name="ones_sb")
            nc.vector.memset(ones_sb[:], 1.0)

            # ---- persistent activations ----
            qT = [qk.tile([HD, S], BF16, name=f"qT{h}") for h in range(GQ)]
            kT = qk.tile([HD, S], BF16, name="kT")
            v_all = qk.tile([128, NKT, HD], BF16, name="v_all")
            wo_sb = [wp.tile([128, DL], BF16, name=f"wo{e}") for e in range(ET)]
            pat_sb = [wp.tile([128, 512], BF16, name=f"pat{i}")
                      for i in range(n_pat)]

            def rope(ps, out_slice, sc):
                cs = slice(sc * 512, (sc + 1) * 512)
                t1 = rp.tile([128, 512], F32, name="rope_t1")
                nc.vector.tensor_mul(t1[0:64, :], ps[64:128, :], sin_sb[0:64, cs])
                nc.vector.tensor_mul(t1[64:128, :], ps[0:64, :], sin_sb[64:128, cs])
                t2 = rp.tile([128, 512], F32, name="rope_t2")
                nc.vector.tensor_mul(t2[:], ps[:], cos_sb[:, cs])
                nc.vector.tensor_add(out_slice, t2[:], t1[:])

            def preload_x(sc):
                cs = slice(sc * 512, (sc + 1) * 512)
                xts = []
                for e in range(ET):
                    t = xp.tile([128, 512], BF16, name=f"xts{e}")
                    nc.sync.dma_start(t[:], xT_d[e * 128:(e + 1) * 128, cs])
                    xts.append(t)
                return xts

            def qkv_gen(sc, xts=None):
                """Yields once per matmul; 6 groups (4 q heads, k, v)."""
                cs = slice(sc * 512, (sc + 1) * 512)
                if xts is None:
                    xts = preload_x(sc)

                def fin_q(h):
                    return lambda ps: rope(ps, qT[h][:, cs], sc)

                def fin_k(ps):
                    rope(ps, kT[:, cs], sc)

                def fin_v(ps):
                    vt = rp.tile([128, 512], BF16, name="vt_tmp")
                    nc.vector.tensor_copy(vt[:], ps[:])
                    for i in range(4):
                        vtp = psp.tile([128, 128], BF16, name="vtp_ps",
                                       tag="sum_ps", bufs=1)
                        nc.tensor.transpose(vtp[:], vt[:, i * 128:(i + 1) * 128],
                                            ident_sb[:])
                        nc.vector.tensor_copy(v_all[:, sc * 4 + i, :], vtp[:])

                def blk_q(h):
                    return (lambda e, h=h: wq_sb[e][:, h * 128:(h + 1) * 128],
                            fin_q(h))

                if sc == 0:
                    blocks = [blk_q(0), (lambda e: wk_sb[e][:], fin_k),
                              blk_q(1), (lambda e: wv_sb[e][:], fin_v),
                              blk_q(2), blk_q(3)]
                else:
                    blocks = [blk_q(0), blk_q(1),
                              (lambda e: wk_sb[e][:], fin_k),
                              (lambda e: wv_sb[e][:], fin_v),
                              blk_q(2), blk_q(3)]
                for lhs_fn, fin in blocks:
                    ps = psp.tile([128, 512], F32, name="fill_ps", bufs=2)
                    for e in range(ET):
                        nc.tensor.matmul(ps[:], lhs_fn(e), xts[e][:],
                                         start=(e == 0), stop=(e == ET - 1))
                        yield
                    fin(ps)

            def oproj_gen(c, ag_outs):
                """Yields once per matmul; 4 groups of 16 (o-proj chunk c).
                Gathered-ctx loads are emitted lazily at the first pull —
                by then (two chunks after the AllGather was issued) the
                collective is long done, so the loads cannot head-of-line
                block the sync DMA queue."""
                tiles = []
                order = []
                g = 0
                for ag_out, itiles in ag_outs:
                    for r0 in range(0, len(itiles), 4):
                        t = fp.tile([128, 4, 512], BF16, name=f"ctxF{g}")
                        nc.sync.dma_start(
                            t[:], ag_out[r0 * 128:(r0 + 4) * 128, :].rearrange(
                                "(a p) f -> p a f", p=128))
                        tiles.append(t)
                        for j in range(4):
                            order.append((g, j, itiles[r0 + j]))
                        g += 1
                for qs in range(4):
                    ops = psp.tile([128, 512], F32, name="fill_ps", bufs=2)
                    for j, (g, jj, i) in enumerate(order):
                        nc.tensor.matmul(
                            ops[:],
                            tiles[g][:, jj, qs * 128:(qs + 1) * 128],
                            wo_sb[i][:],
                            start=(j == 0), stop=(j == ET - 1))
                        yield
                    osb = op.tile([128, 512], F32, name="osb")
                    nc.vector.tensor_copy(osb[:], ops[:])
                    r0 = c * 512 + qs * 128
                    nc.sync.dma_start(out_d[r0:r0 + 128, :], osb[:])

            carry = []

            def emit_attn(c, feeder, n_fill, split_ag=False, defer_tail=False):
                """Attention for q chunk c in transposed-score form. With
                defer_tail, the last head's sums/MM2 and the AllGather issue
                are executed early in the NEXT chunk's emission so the chunk
                boundary never stalls TensorE."""
                kts = vis_kts[c]
                cs = slice(c * 512, (c + 1) * 512)
                n_attn_mm = len(kts) * GQ * 3  # MM1T + ones + MM2 per (h, kt)
                ratio = n_fill / max(1, n_attn_mm)
                acc = 0.0

                def tick(k=1):
                    nonlocal acc
                    acc += ratio * k
                    n = int(acc)
                    if n:
                        feeder.pull(n)
                        acc -= n

                if split_ag:
                    ag_ins = [dr.tile([2 * HD, 512], BF16, name="ag_in_a"),
                              dr.tile([2 * HD, 512], BF16, name="ag_in_b")]
                else:
                    ag_ins = [dr.tile([DL, 512], BF16, name="ag_in")]
                ags = []

                def issue_ag(part):
                    ag_in = ag_ins[part]
                    nr = ag_in.shape[0]
                    ag_out = dr.tile([4 * nr, 512], BF16,
                                     name=f"ag_out{'ab'[part] if split_ag else ''}")
                    nc.gpsimd.collective_compute(
                        "AllGather", mybir.AluOpType.bypass,
                        replica_groups=[[0, 1, 2, 3], [4, 5, 6, 7]],
                        ins=[ag_in[:].opt()], outs=[ag_out[:].opt()])
                    nh_part = nr // HD
                    itiles = [4 * r + part * nh_part + h
                              for r in range(4) for h in range(nh_part)]
                    ags.append((ag_out, itiles))

                done_mm2 = [0]
                pending = []
                for h in range(GQ):
                    PT = ptp.tile([128, NKT, 512], BF16, name="PT")
                    ksums = []
                    flush_carry = carry and h == 0
                    # MM1T per visible k-tile; exp over adjacent-kt pairs
                    for i0 in range(0, len(kts), 2):
                        pair = kts[i0:i0 + 2]
                        ps = psp.tile([128, 1024], F32, name="mm1_ps", bufs=2)
                        for j, kt in enumerate(pair):
                            nc.tensor.matmul(ps[:, j * 512:(j + 1) * 512],
                                             kT[:, kt * 128:(kt + 1) * 128],
                                             qT[h][:, cs])
                            tick()
                        if len(pair) == 2 and pair[1] == pair[0] + 1:
                            nc.scalar.activation(
                                PT[:, pair[0]:pair[0] + 2, :],
                                ps[:, 0:1024], EXP, scale=INV_SQRT_HD)
                        else:
                            for j, kt in enumerate(pair):
                                nc.scalar.activation(
                                    PT[:, kt, :], ps[:, j * 512:(j + 1) * 512],
                                    EXP, scale=INV_SQRT_HD)
                        for kt in pair:
                            code = vis[c][kt]
                            if code >= 2:
                                nc.vector.tensor_mul(PT[:, kt, :], PT[:, kt, :],
                                                     pat_sb[code - 2][:])
                        # pre-combine groups of 4 k-tiles for the row sums on
                        # VectorE; overlaps the later MM1T pairs on TensorE
                        if i0 % 8 == 0:
                            ksums.append([None, kts[i0:i0 + 8]])
                            if len(pair) == 2:
                                ksum = sp.tile([128, 512], BF16, name="ktsum",
                                               bufs=6)
                                nc.vector.tensor_add(ksum[:], PT[:, pair[0], :],
                                                     PT[:, pair[1], :])
                                ksums[-1][0] = ksum
                        else:
                            ksum = ksums[-1][0]
                            for kt in pair:
                                nc.vector.tensor_add(ksum[:], ksum[:],
                                                     PT[:, kt, :])

                    def finish(h=h, PT=PT, hks=list(ksums)):
                        # row sums via ones-matmul per combined group, then
                        # 1/sum broadcast; MM2; normalize; AllGather input
                        sum_ps = psp.tile([1, 512], F32, name="sum_ps",
                                          tag="sum_ps", bufs=1)
                        for gi, (ksum, grp) in enumerate(hks):
                            rhs = ksum[:] if ksum is not None else PT[:, grp[0], :]
                            nc.tensor.matmul(sum_ps[:], ones_sb[:], rhs,
                                             start=(gi == 0),
                                             stop=(gi == len(hks) - 1))
                            tick()
                        rrow = sp.tile([1, 512], F32, name="rrow", bufs=2)
                        nc.vector.reciprocal(rrow[:], sum_ps[:])
                        rbc = sp.tile([128, 512], F32, name="rbc", bufs=2)
                        nc.gpsimd.partition_broadcast(rbc[:], rrow[0:1, :])
                        ctx_ps = psp.tile([128, 512], F32, name="ctx_ps",
                                          tag="ctx_ps", bufs=1)
                        for j, kt in enumerate(kts):
                            nc.tensor.matmul(ctx_ps[:], v_all[:, kt, :],
                                             PT[:, kt, :], start=(j == 0),
                                             stop=(j == len(kts) - 1))
                            tick()
                        ctx_sb = cp.tile([128, 512], BF16, name=f"ctx{h}")
                        nc.vector.tensor_mul(ctx_sb[:], ctx_ps[:], rbc[:])
                        if split_ag:
                            ag_in, row = ag_ins[h // 2], (h % 2) * 128
                        else:
                            ag_in, row = ag_ins[0], h * 128
                        nc.sync.dma_start(ag_in[row:row + 128, :], ctx_sb[:])
                        done_mm2[0] += 1
                        if split_ag and done_mm2[0] == 2:
                            issue_ag(0)

                    if flush_carry:
                        carry.pop(0)()
                    pending.append(finish)
                    if len(pending) > 1:
                        pending.pop(0)()
                feeder.pull(8)
                if defer_tail and not split_ag:
                    last = pending.pop()

                    def tail(last=last):
                        last()
                        issue_ag(0)
                    carry.append(tail)
                else:
                    while pending:
                        pending.pop(0)()
                    if split_ag:
                        issue_ag(1)
                    else:
                        issue_ag(0)
                return ags

            # ---- pipelined emission ----
            f0 = Feeder()
            f0.add(qkv_gen(0, xts0))
            f0.drain()
            for sc in range(1, NSC):
                cs = slice(sc * 512, (sc + 1) * 512)
                nc.scalar.dma_start(cos_sb[:, cs], cosT_d[:, cs])
                nc.scalar.dma_start(sin_sb[:, cs], sinT_d[:, cs])
            for i in range(n_pat):
                nc.scalar.dma_start(pat_sb[i][:], pats_d[i])
            for e in range(ET):
                nc.scalar.dma_start(wo_sb[e][:], woT_d[e * 128:(e + 1) * 128, :])

            fd = Feeder()
            fd.add(qkv_gen(1))
            fd.add(qkv_gen(2))
            ags0 = emit_attn(0, fd, 192, defer_tail=True)
            fd.add(qkv_gen(3))
            ags1 = emit_attn(1, fd, 96)
            fd.add(oproj_gen(0, ags0))
            ags2 = emit_attn(2, fd, 64)
            fd.add(oproj_gen(1, ags1))
            fd.add(oproj_gen(2, ags2))
            ags3 = emit_attn(3, fd, 128, split_ag=True)
            fd.add(oproj_gen(3, ags3))
            fd.drain()

    nc.compile()
    return nc


def _prepare(hidden_states, wq, wk, wv, wo, attention_mask, position_ids):
    hidden_states = np.asarray(hidden_states, dtype=np.float32)
    wq = np.asarray(wq, dtype=np.float32)
    wk = np.asarray(wk, dtype=np.float32)
    wv = np.asarray(wv, dtype=np.float32)
    wo = np.asarray(wo, dtype=np.float32)
    pos = np.asarray(position_ids)

    vis, pats, n_pat = _classify_mask(attention_mask)
    key = (tuple(tuple(r) for r in vis), n_pat)
    if key not in _cache:
        _cache[key] = _build(vis, n_pat)
    nc = _cache[key]

    # RoPE tables per batch: cosT/sinT [HD, S]; sinT sign-folded (-sin for d<64)
    inv_freq = 1.0 / (ROPE_THETA ** (np.arange(0, HD, 2, dtype=np.float32) / HD))
    cosT = np.empty((B, HD, S), np.float32)
    sinT = np.empty((B, HD, S), np.float32)
    for b in range(B):
        freqs = pos[b].astype(np.float32)[None, :] * inv_freq[:, None]  # [64, S]
        cosT[b] = np.concatenate([np.cos(freqs)] * 2, axis=0)
        sn = np.sin(freqs)
        sinT[b] = np.concatenate([-sn, sn], axis=0)

    bf = ml_dtypes.bfloat16
    xT = [np.ascontiguousarray(hidden_states[b].T).astype(bf) for b in range(B)]
    ident = np.eye(128, dtype=np.float32).astype(bf)
    in_maps = []
    for c in range(8):
        b, g = c // 4, c % 4
        in_maps.append({
            "xT": xT[b],
            "wqT": np.ascontiguousarray(wq[g * DL:(g + 1) * DL, :].T).astype(bf),
            "wkT": np.ascontiguousarray(wk[g * HD:(g + 1) * HD, :].T).astype(bf),
            "wvT": np.ascontiguousarray(wv[g * HD:(g + 1) * HD, :].T).astype(bf),
            "woT": np.ascontiguousarray(wo[g * DL:(g + 1) * DL, :].T).astype(bf),
            "cosT": cosT[b],
            "sinT": sinT[b],
            "pats": pats[b].astype(bf),
            "ident": ident,
        })
    return nc, in_maps


def kernel(hidden_states, wq, wk, wv, wo, attention_mask, position_ids):
    nc, in_maps = _prepare(hidden_states, wq, wk, wv, wo, attention_mask,
                           position_ids)
    res = run_bass_kernel_spmd(nc, in_maps, core_ids=list(range(8))).results
    out = np.empty((B, S, H), np.float32)
    for c in range(8):
        b, g = c // 4, c % 4
        out[b, :, g * DL:(g + 1) * DL] = res[c]["out"]
    return out


# revision 64
# speedup vs baseline: 1.0586x; 1.0382x over previous
"""GQA attention layer (B=2, S=2048, H=2048, 16 q heads / 4 kv heads, RoPE, causal
mask) on 8 TRN2 NeuronCores.

Sharding: core c = (b, g) with b = c // 4 (batch), g = c % 4 (kv-head group).
Each core computes q-heads 4g..4g+3 and kv-head g for batch b:
  - QKV projections from x^T (bf16 matmuls, f32 PSUM accumulate)
  - RoPE on q/k (f32, host-precomputed cos/sin tables); v transposed via PE
  - MM1 computes TRANSPOSED scores S^T[k, q] (lhsT = k^T tile, rhs = q^T), so
    exp on ScalarE writes P^T directly -- no DMA transposes anywhere (Tile
    hard-serializes DMA transposes against collectives, which would freeze
    the pipeline during every AllGather)
  - row-sums of P: groups of 4 k-tiles combined on VectorE, then a
    ones-vector matmul; 1/sum applied to ctx^T via GpSimd partition
    broadcast + VectorE multiply
  - MM2: ctx^T[d, q] = sum_kt v[kt]^T-block @ P^T[kt]
  - AllGather ctx^T across the 4 cores of the batch (groups [[0..3],[4..7]])
  - o-proj: each core computes its 512-column block of the output for all S.
Host reassembles [B, S, H] from the 8 [S, 512] column blocks.

The additive attention mask is handled generally: each 512q x 128k score tile
is classified at build time (from the actual mask input) as fully-masked
(matmuls skipped), zero (no mask op), or mixed (P^T is multiplied post-exp
by a per-core transposed exp(mask) tile). For the causal mask this skips the upper
triangle (~half the attention FLOPs) and needs only 4 distinct patterns.

Scheduling: the attention chain (MM1 -> exp -> sums -> MM2) is latency- and
ScalarE-bound; independent matmul work (QKV projection of later chunks,
o-proj of earlier chunks) is fed into the emission stream at single-matmul
granularity between the dependent attention matmuls, keeping TensorE dense.
AllGather-gated DMA (gathered-ctx loads, output writes) lives on the GpSimd
SWDGE queue so it cannot head-of-line-block compute-critical DMA queues.
"""
import math
import os
import sys

for _p in ("/opt/trn_rl_repo",):
    if _p not in sys.path and os.path.isdir(_p):
        sys.path.insert(0, _p)

import ml_dtypes
import numpy as np

from concourse import bacc, mybir, tile
from concourse.bass_utils import run_bass_kernel_spmd

BF16 = mybir.dt.bfloat16
F32 = mybir.dt.float32
EXP = mybir.ActivationFunctionType.Exp

B, S, H = 2, 2048, 2048
NH, NKV, HD = 16, 4, 128
GQ = NH // NKV            # q heads per core (4)
DL = GQ * HD              # local q width (512)
ET = H // 128             # e-tiles (16)
NSC = S // 512            # 512-wide s/q chunks (4)
NKT = S // 128            # 128-row k tiles (16)
ROPE_THETA = 10000.0
INV_SQRT_HD = 1.0 / math.sqrt(HD)
SQRT_HD = math.sqrt(HD)

SKIP, FREE = 0, 1         # vis codes; >=2 means pattern index (code - 2)

_cache = {}


def _classify_mask(mask):
    """mask: [B, 1, S, S] f32 additive. Tiles are [512 q x 128 k] per
    (chunk c, k-tile kt). Returns (vis, pats): vis[c][kt] in
    {SKIP, FREE, idx+2}; pats[b] is [n_pat, 128, 512] f32 — TRANSPOSED
    [k, q] patterns, clipped and pre-multiplied by sqrt(HD)."""
    m = np.asarray(mask, dtype=np.float32).reshape(B, S, S)
    vis = [[FREE] * NKT for _ in range(NSC)]
    pat_ids = {}
    pats = [[] for _ in range(B)]
    for c in range(NSC):
        for kt in range(NKT):
            blk = m[:, c * 512:(c + 1) * 512, kt * 128:(kt + 1) * 128]
            if np.all(blk <= -1e8):
                vis[c][kt] = SKIP
            elif np.all(blk == 0.0):
                vis[c][kt] = FREE
            else:
                # multiplicative post-exp pattern (transposed [k, q]):
                # exp(s + m) == exp(s) * exp(m)
                factor = np.exp(np.maximum(blk, -90.0)).transpose(0, 2, 1)
                factor = np.ascontiguousarray(factor)
                key = factor.tobytes()
                if key not in pat_ids:
                    pat_ids[key] = len(pats[0])
                    for b in range(B):
                        pats[b].append(factor[b])
                vis[c][kt] = pat_ids[key] + 2
    n_pat = len(pats[0])
    if n_pat == 0:
        pats_np = [np.zeros((1, 128, 512), np.float32) for _ in range(B)]
        n_pat = 1
    else:
        pats_np = [np.stack(p) for p in pats]
    return vis, pats_np, n_pat


class Feeder:
    """FIFO pull of single-matmul filler steps from generators."""

    def __init__(self):
        self.gens = []

    def add(self, gen):
        self.gens.append(gen)

    def pull(self, n):
        while n > 0 and self.gens:
            try:
                next(self.gens[0])
                n -= 1
            except StopIteration:
                self.gens.pop(0)

    def drain(self):
        while self.gens:
            self.pull(1 << 30)


def _build(vis, n_pat):
    nc = bacc.Bacc(None, target_bir_lowering=False, num_devices=8)

    xT_d = nc.dram_tensor("xT", [H, S], BF16, kind="ExternalInput")
    wqT_d = nc.dram_tensor("wqT", [H, DL], BF16, kind="ExternalInput")
    wkT_d = nc.dram_tensor("wkT", [H, HD], BF16, kind="ExternalInput")
    wvT_d = nc.dram_tensor("wvT", [H, HD], BF16, kind="ExternalInput")
    woT_d = nc.dram_tensor("woT", [H, DL], BF16, kind="ExternalInput")
    cosT_d = nc.dram_tensor("cosT", [HD, S], F32, kind="ExternalInput")
    sinT_d = nc.dram_tensor("sinT", [HD, S], F32, kind="ExternalInput")
    pats_d = nc.dram_tensor("pats", [n_pat, 128, 512], BF16, kind="ExternalInput")
    ident_d = nc.dram_tensor("ident", [128, 128], BF16, kind="ExternalInput")
    out_d = nc.dram_tensor("out", [S, DL], F32, kind="ExternalOutput")

    vis_kts = [[kt for kt in range(NKT) if vis[c][kt] != SKIP] for c in range(NSC)]

    with tile.TileContext(nc) as tc:
        with (
            tc.tile_pool(name="wp", bufs=1) as wp,
            tc.tile_pool(name="xp", bufs=2) as xp,
            tc.tile_pool(name="qk", bufs=1) as qk,
            tc.tile_pool(name="rp", bufs=2) as rp,
            tc.tile_pool(name="ptp", bufs=2) as ptp,
            tc.tile_pool(name="cp", bufs=2) as cp,
            tc.tile_pool(name="fp", bufs=1) as fp,
            tc.tile_pool(name="op", bufs=2) as op,
            tc.tile_pool(name="sp", bufs=4) as sp,
            tc.tile_pool(name="ps", bufs=1, space="PSUM") as psp,
            tc.tile_pool(name="dr", bufs=2, space="DRAM") as dr,
        ):
            # ---- weights / tables for phase 1 ----
            wq_sb = [wp.tile([128, DL], BF16, name=f"wq{e}") for e in range(ET)]
            wk_big = wp.tile([128, ET, HD], BF16, name="wk_big")
            wv_big = wp.tile([128, ET, HD], BF16, name="wv_big")
            wk_sb = [wk_big[:, e, :] for e in range(ET)]
            wv_sb = [wv_big[:, e, :] for e in range(ET)]
            cos_sb = wp.tile([HD, S], F32, name="cos_sb")
            sin_sb = wp.tile([HD, S], F32, name="sin_sb")
            nc.scalar.dma_start(cos_sb[:, 0:512], cosT_d[:, 0:512])
            nc.scalar.dma_start(sin_sb[:, 0:512], sinT_d[:, 0:512])
            nc.gpsimd.dma_start(
                wk_big[:], wkT_d[:].rearrange("(a p) f -> p a f", p=128))
            nc.gpsimd.dma_start(
                wv_big[:], wvT_d[:].rearrange("(a p) f -> p a f", p=128))
            xts0 = []
            for e in range(ET):
                r = slice(e * 128, (e + 1) * 128)
                nc.sync.dma_start(wq_sb[e][:], wqT_d[r, :])
                t = xp.tile([128, 512], BF16, name=f"xts{e}")
                nc.scalar.dma_start(t[:], xT_d[r, 0:512])
                xts0.append(t)
            ident_sb = wp.tile([128, 128], BF16, name="ident_sb")
            nc.scalar.dma_start(ident_sb[:], ident_d[:])
            ones_sb = wp.tile([128, 1], BF16, name="ones_sb")
            nc.vector.memset(ones_sb[:], 1.0)

            # ---- persistent activations ----
            qT = [qk.tile([HD, S], BF16, name=f"qT{h}") for h in range(GQ)]
            kT = qk.tile([HD, S], BF16, name="kT")
            v_all = qk.tile([128, NKT, HD], BF16, name="v_all")
            wo_sb = [wp.tile([128, DL], BF16, name=f"wo{e}") for e in range(ET)]
            pat_sb = [wp.tile([128, 512], BF16, name=f"pat{i}")
                      for i in range(n_pat)]

            def rope(ps, out_slice, sc):
                cs = slice(sc * 512, (sc + 1) * 512)
                t1 = rp.tile([128, 512], F32, name="rope_t1")
                nc.vector.tensor_mul(t1[0:64, :], ps[64:128, :], sin_sb[0:64, cs])
                nc.vector.tensor_mul(t1[64:128, :], ps[0:64, :], sin_sb[64:128, cs])
                t2 = rp.tile([128, 512], F32, name="rope_t2")
                nc.vector.tensor_mul(t2[:], ps[:], cos_sb[:, cs])
                nc.vector.tensor_add(out_slice, t2[:], t1[:])

            def preload_x(sc):
                cs = slice(sc * 512, (sc + 1) * 512)
                xts = []
                for e in range(ET):
                    t = xp.tile([128, 512], BF16, name=f"xts{e}")
                    nc.sync.dma_start(t[:], xT_d[e * 128:(e + 1) * 128, cs])
                    xts.append(t)
                return xts

            def qkv_gen(sc, xts=None):
                """Yields once per matmul; 6 groups (4 q heads, k, v)."""
                cs = slice(sc * 512, (sc + 1) * 512)
                if xts is None:
                    xts = preload_x(sc)

                def fin_q(h):
                    return lambda ps: rope(ps, qT[h][:, cs], sc)

                def fin_k(ps):
                    rope(ps, kT[:, cs], sc)

                def fin_v(ps):
                    vt = rp.tile([128, 512], BF16, name="vt_tmp")
                    nc.vector.tensor_copy(vt[:], ps[:])
                    for i in range(4):
                        vtp = psp.tile([128, 128], BF16, name="vtp_ps",
                                       tag="sum_ps", bufs=1)
                        nc.tensor.transpose(vtp[:], vt[:, i * 128:(i + 1) * 128],
                                            ident_sb[:])
                        nc.vector.tensor_copy(v_all[:, sc * 4 + i, :], vtp[:])

                def blk_q(h):
                    return (lambda e, h=h: wq_sb[e][:, h * 128:(h + 1) * 128],
                            fin_q(h))

                if sc == 0:
                    blocks = [blk_q(0), (lambda e: wk_sb[e], fin_k),
                              blk_q(1), (lambda e: wv_sb[e], fin_v),
                              blk_q(2), blk_q(3)]
                else:
                    blocks = [blk_q(0), blk_q(1),
                              (lambda e: wk_sb[e], fin_k),
                              (lambda e: wv_sb[e], fin_v),
                              blk_q(2), blk_q(3)]
                for lhs_fn, fin in blocks:
                    ps = psp.tile([128, 512], F32, name="fill_ps", bufs=2)
                    for e in range(ET):
                        nc.tensor.matmul(ps[:], lhs_fn(e), xts[e][:],
                                         start=(e == 0), stop=(e == ET - 1))
                        yield
                    fin(ps)

            def oproj_gen(c, ag_outs):
                """Yields once per matmul; 4 groups of 16 (o-proj chunk c).
                Gathered-ctx loads are emitted lazily at the first pull —
                by then (two chunks after the AllGather was issued) the
                collective is long done, so the loads cannot head-of-line
                block the sync DMA queue."""
                tiles = []
                order = []
                g = 0
                for ag_out, itiles in ag_outs:
                    for r0 in range(0, len(itiles), 4):
                        t = fp.tile([128, 4, 512], BF16, name=f"ctxF{g}")
                        nc.sync.dma_start(
                            t[:], ag_out[r0 * 128:(r0 + 4) * 128, :].rearrange(
                                "(a p) f -> p a f", p=128))
                        tiles.append(t)
                        for j in range(4):
                            order.append((g, j, itiles[r0 + j]))
                        g += 1
                for qs in range(4):
                    ops = psp.tile([128, 512], F32, name="fill_ps", bufs=2)
                    for j, (g, jj, i) in enumerate(order):
                        nc.tensor.matmul(
                            ops[:],
                            tiles[g][:, jj, qs * 128:(qs + 1) * 128],
                            wo_sb[i][:],
                            start=(j == 0), stop=(j == ET - 1))
                        yield
                    osb = op.tile([128, 512], F32, name="osb")
                    nc.vector.tensor_copy(osb[:], ops[:])
                    r0 = c * 512 + qs * 128
                    nc.sync.dma_start(out_d[r0:r0 + 128, :], osb[:])

            carry = []

            def emit_attn(c, feeder, n_fill, split_ag=False, defer_tail=False):
                """Attention for q chunk c in transposed-score form. With
                defer_tail, the last head's sums/MM2 and the AllGather issue
                are executed early in the NEXT chunk's emission so the chunk
                boundary never stalls TensorE."""
                kts = vis_kts[c]
                cs = slice(c * 512, (c + 1) * 512)
                n_attn_mm = len(kts) * GQ * 3  # MM1T + ones + MM2 per (h, kt)
                ratio = n_fill / max(1, n_attn_mm)
                acc = 0.0

                def tick(k=1):
                    nonlocal acc
                    acc += ratio * k
                    n = int(acc)
                    if n:
                        feeder.pull(n)
                        acc -= n

                if split_ag:
                    ag_ins = [dr.tile([2 * HD, 512], BF16, name="ag_in_a"),
                              dr.tile([2 * HD, 512], BF16, name="ag_in_b")]
                else:
                    ag_ins = [dr.tile([DL, 512], BF16, name="ag_in")]
                ags = []

                def issue_ag(part):
                    ag_in = ag_ins[part]
                    nr = ag_in.shape[0]
                    ag_out = dr.tile([4 * nr, 512], BF16,
                                     name=f"ag_out{'ab'[part] if split_ag else ''}")
                    nc.gpsimd.collective_compute(
                        "AllGather", mybir.AluOpType.bypass,
                        replica_groups=[[0, 1, 2, 3], [4, 5, 6, 7]],
                        ins=[ag_in[:].opt()], outs=[ag_out[:].opt()])
                    nh_part = nr // HD
                    itiles = [4 * r + part * nh_part + h
                              for r in range(4) for h in range(nh_part)]
                    ags.append((ag_out, itiles))

                done_mm2 = [0]
                pending = []
                for h in range(GQ):
                    PT = ptp.tile([128, NKT, 512], BF16, name="PT")
                    ksums = []
                    flush_carry = carry and h == 0
                    # MM1T per visible k-tile; exp over adjacent-kt pairs
                    for i0 in range(0, len(kts), 2):
                        pair = kts[i0:i0 + 2]
                        ps = psp.tile([128, 1024], F32, name="mm1_ps", bufs=2)
                        for j, kt in enumerate(pair):
                            nc.tensor.matmul(ps[:, j * 512:(j + 1) * 512],
                                             kT[:, kt * 128:(kt + 1) * 128],
                                             qT[h][:, cs])
                            tick()
                        if len(pair) == 2 and pair[1] == pair[0] + 1:
                            nc.scalar.activation(
                                PT[:, pair[0]:pair[0] + 2, :],
                                ps[:, 0:1024], EXP, scale=INV_SQRT_HD)
                        else:
                            for j, kt in enumerate(pair):
                                nc.scalar.activation(
                                    PT[:, kt, :], ps[:, j * 512:(j + 1) * 512],
                                    EXP, scale=INV_SQRT_HD)
                        for kt in pair:
                            code = vis[c][kt]
                            if code >= 2:
                                nc.vector.tensor_mul(PT[:, kt, :], PT[:, kt, :],
                                                     pat_sb[code - 2][:])
                        # pre-combine groups of 4 k-tiles for the row sums on
                        # VectorE; overlaps the later MM1T pairs on TensorE
                        if i0 % 8 == 0:
                            ksums.append([None, kts[i0:i0 + 8]])
                            if len(pair) == 2:
                                ksum = sp.tile([128, 512], BF16, name="ktsum",
                                               bufs=6)
                                nc.vector.tensor_add(ksum[:], PT[:, pair[0], :],
                                                     PT[:, pair[1], :])
                                ksums[-1][0] = ksum
                        else:
                            ksum = ksums[-1][0]
                            for kt in pair:
                                nc.vector.tensor_add(ksum[:], ksum[:],
                                                     PT[:, kt, :])

                    def finish(h=h, PT=PT, hks=list(ksums)):
                        # row sums via ones-matmul per combined group, then
                        # 1/sum broadcast; MM2; normalize; AllGather input
                        sum_ps = psp.tile([1, 512], F32, name="sum_ps",
                                          tag="sum_ps", bufs=1)
                        for gi, (ksum, grp) in enumerate(hks):
                            rhs = ksum[:] if ksum is not None else PT[:, grp[0], :]
                            nc.tensor.matmul(sum_ps[:], ones_sb[:], rhs,
                                             start=(gi == 0),
                                             stop=(gi == len(hks) - 1))
                            tick()
                        rrow = sp.tile([1, 512], F32, name="rrow", bufs=2)
                        nc.vector.reciprocal(rrow[:], sum_ps[:])
                        rbc = sp.tile([128, 512], F32, name="rbc", bufs=2)
                        nc.gpsimd.partition_broadcast(rbc[:], rrow[0:1, :])
                        ctx_ps = psp.tile([128, 512], F32, name="ctx_ps",
                                          tag="ctx_ps", bufs=1)
                        for j, kt in enumerate(kts):
                            nc.tensor.matmul(ctx_ps[:], v_all[:, kt, :],
                                             PT[:, kt, :], start=(j == 0),
                                             stop=(j == len(kts) - 1))
                            tick()
                        ctx_sb = cp.tile([128, 512], BF16, name=f"ctx{h}")
                        nc.vector.tensor_mul(ctx_sb[:], ctx_ps[:], rbc[:])
                        if split_ag:
                            ag_in, row = ag_ins[h // 2], (h % 2) * 128
                        else:
                            ag_in, row = ag_ins[0], h * 128
                        nc.sync.dma_start(ag_in[row:row + 128, :], ctx_sb[:])
                        done_mm2[0] += 1
                        if split_ag and done_mm2[0] == 2:
                            issue_ag(0)

                    if flush_carry:
                        carry.pop(0)()
                    pending.append(finish)
                    if len(pending) > 1:
                        pending.pop(0)()
                feeder.pull(8)
                if defer_tail and not split_ag:
                    last = pending.pop()

                    def tail(last=last):
                        last()
                        issue_ag(0)
                    carry.append(tail)
                else:
                    while pending:
                        pending.pop(0)()
                    if split_ag:
                        issue_ag(1)
                    else:
                        issue_ag(0)
                return ags

            # ---- pipelined emission ----
            f0 = Feeder()
            f0.add(qkv_gen(0, xts0))
            f0.drain()
            for sc in range(1, NSC):
                cs = slice(sc * 512, (sc + 1) * 512)
                nc.scalar.dma_start(cos_sb[:, cs], cosT_d[:, cs])
                nc.scalar.dma_start(sin_sb[:, cs], sinT_d[:, cs])
            for i in range(n_pat):
                nc.scalar.dma_start(pat_sb[i][:], pats_d[i])
            for e in range(ET):
                nc.scalar.dma_start(wo_sb[e][:], woT_d[e * 128:(e + 1) * 128, :])

            fd = Feeder()
            fd.add(qkv_gen(1))
            fd.add(qkv_gen(2))
            ags0 = emit_attn(0, fd, 192, defer_tail=True)
            fd.add(qkv_gen(3))
            ags1 = emit_attn(1, fd, 96)
            fd.add(oproj_gen(0, ags0))
            ags2 = emit_attn(2, fd, 64)
            fd.add(oproj_gen(1, ags1))
            fd.add(oproj_gen(2, ags2))
            ags3 = emit_attn(3, fd, 128, split_ag=True)
            fd.add(oproj_gen(3, ags3))
            fd.drain()

    nc.compile()
    return nc


def _prepare(hidden_states, wq, wk, wv, wo, attention_mask, position_ids):
    hidden_states = np.asarray(hidden_states, dtype=np.float32)
    wq = np.asarray(wq, dtype=np.float32)
    wk = np.asarray(wk, dtype=np.float32)
    wv = np.asarray(wv, dtype=np.float32)
    wo = np.asarray(wo, dtype=np.float32)
    pos = np.asarray(position_ids)

    vis, pats, n_pat = _classify_mask(attention_mask)
    key = (tuple(tuple(r) for r in vis), n_pat)
    if key not in _cache:
        _cache[key] = _build(vis, n_pat)
    nc = _cache[key]

    # RoPE tables per batch: cosT/sinT [HD, S]; sinT sign-folded (-sin for d<64)
    inv_freq = 1.0 / (ROPE_THETA ** (np.arange(0, HD, 2, dtype=np.float32) / HD))
    cosT = np.empty((B, HD, S), np.float32)
    sinT = np.empty((B, HD, S), np.float32)
    for b in range(B):
        freqs = pos[b].astype(np.float32)[None, :] * inv_freq[:, None]  # [64, S]
        cosT[b] = np.concatenate([np.cos(freqs)] * 2, axis=0)
        sn = np.sin(freqs)
        sinT[b] = np.concatenate([-sn, sn], axis=0)

    bf = ml_dtypes.bfloat16
    xT = [np.ascontiguousarray(hidden_states[b].T).astype(bf) for b in range(B)]
    ident = np.eye(128, dtype=np.float32).astype(bf)
    in_maps = []
    for c in range(8):
        b, g = c // 4, c % 4
        in_maps.append({
            "xT": xT[b],
            "wqT": np.ascontiguousarray(wq[g * DL:(g + 1) * DL, :].T).astype(bf),
            "wkT": np.ascontiguousarray(wk[g * HD:(g + 1) * HD, :].T).astype(bf),
            "wvT": np.ascontiguousarray(wv[g * HD:(g + 1) * HD, :].T).astype(bf),
            "woT": np.ascontiguousarray(wo[g * DL:(g + 1) * DL, :].T).astype(bf),
            "cosT": cosT[b],
            "sinT": sinT[b],
            "pats": pats[b].astype(bf),
            "ident": ident,
        })
    return nc, in_maps


def kernel(hidden_states, wq, wk, wv, wo, attention_mask, position_ids):
    nc, in_maps = _prepare(hidden_states, wq, wk, wv, wo, attention_mask,
                           position_ids)
    res = run_bass_kernel_spmd(nc, in_maps, core_ids=list(range(8))).results
    out = np.empty((B, S, H), np.float32)
    for c in range(8):
        b, g = c // 4, c % 4
        out[b, :, g * DL:(g + 1) * DL] = res[c]["out"]
    return out
